# revision 33
# baseline (speedup 1.0000x reference)
"""Trainium2 Bass kernel for a dense transformer decoder layer.

B=2, L=2048, E=1024, H=16 (Dh=64), Dff=4096, fp32 I/O.

Strategy (8 NeuronCores):
  - Sequence parallel: B*L = 4096 tokens sharded 512 rows/core using a
    zigzag assignment of 128-row blocks {q, 7-q, 8+q, 15-q} within each
    batch (4 cores per batch) so causal attention work is balanced and
    the program is identical on every core (q-tile key extents padded to
    the uniform {512, 1024, 1536, 2048}; the per-core causal boundary is
    data-driven via an additive -1e9 mask on the diagonal 512-chunk).
  - Each core computes K,V for its own rows (all heads), AllGathers them
    in fp16 within its batch group ([[0..3],[4..7]]), then runs causal
    attention for its own query rows, followed by Wo, LN1, FF, LN2 —
    all fully local (no all-reduce anywhere).
  - Big matmuls run as float32r (RNE-rounded to 11 mantissa bits on
    host / by the producing engine): 1 PE cycle/row, ~1.4e-4 rel err.
    Attention score/z matmuls and the W2 matmul run in fp16.
"""

import sys

if "/opt/trn_rl_repo" not in sys.path:
    sys.path.insert(0, "/opt/trn_rl_repo")

from contextlib import ExitStack

import numpy as np

import concourse.bass as bass
import concourse.mybir as mybir
from concourse import bacc
from concourse.bass import ts
from concourse.bass_utils import run_bass_kernel_spmd
from concourse.tile import TileContext

B, L, E, H, Dh, Dff = 2, 2048, 1024, 16, 64, 4096
P = 128
ET = E // P            # 8 feature tiles
FT = Dff // P          # 32 ff tiles
QT = 4                 # q-tiles (128 rows) per core
NCORE = 8
GROUPS = [[0, 1, 2, 3], [4, 5, 6, 7]]
F32, F32R, F16 = mybir.dt.float32, mybir.dt.float32r, mybir.dt.float16
AF = mybir.ActivationFunctionType
OP = mybir.AluOpType
AX = mybir.AxisListType


def _bmap(q):
    return [q, 7 - q, 8 + q, 15 - q]


def _owner(nb):
    if nb < 4:
        return nb, 0
    if nb < 8:
        return 7 - nb, 1
    if nb < 12:
        return nb - 8, 2
    return 15 - nb, 3


def _round_f32r(a):
    b = np.ascontiguousarray(a, np.float32).view(np.uint32)
    r = (b + np.uint32(0x7FF) + ((b >> np.uint32(12)) & np.uint32(1))) & np.uint32(0xFFFFF000)
    return r.view(np.float32)


def _build_program(collectives=True):
    nc = bacc.Bacc("TRN2", target_bir_lowering=False, debug=False, num_devices=NCORE)

    xloc = nc.dram_tensor("xloc", [QT, P, E], F32, kind="ExternalInput")
    xt16 = nc.dram_tensor("xt16", [P, ET, 512], F16, kind="ExternalInput")
    wq = nc.dram_tensor("wq", [E, E], F16, kind="ExternalInput")
    wk = nc.dram_tensor("wk", [E, E], F16, kind="ExternalInput")
    wv = nc.dram_tensor("wv", [E, E], F16, kind="ExternalInput")
    wo = nc.dram_tensor("wo", [E, E], F16, kind="ExternalInput")
    w1 = nc.dram_tensor("w1", [E, Dff], F16, kind="ExternalInput")
    w2 = nc.dram_tensor("w2", [Dff, E], F16, kind="ExternalInput")
    bq2 = nc.dram_tensor("bq2", [P, ET], F32, kind="ExternalInput")
    bk2 = nc.dram_tensor("bk2", [P, ET], F32, kind="ExternalInput")
    bv2 = nc.dram_tensor("bv2", [P, ET], F32, kind="ExternalInput")
    c1l = nc.dram_tensor("c1l", [P, FT], F32, kind="ExternalInput")
    bcast = nc.dram_tensor("bcast", [P, 6, E], F32, kind="ExternalInput")
    maskt = nc.dram_tensor("maskt", [QT, P, 512], F32, kind="ExternalInput")
    id32 = nc.dram_tensor("id32", [P, P], F32, kind="ExternalInput")
    id16 = nc.dram_tensor("id16", [P, P], F16, kind="ExternalInput")
    ones64 = nc.dram_tensor("ones64", [1, 64], F32, kind="ExternalInput")
    yloc = nc.dram_tensor("yloc", [QT, P, E], F32, kind="ExternalOutput")

    with TileContext(nc) as tc, ExitStack() as ctx:
        pp = ctx.enter_context(tc.tile_pool(name="persist", bufs=1))
        dram = ctx.enter_context(tc.tile_pool(name="dram", bufs=1, space="DRAM"))
        lnscr = ctx.enter_context(tc.tile_pool(name="lnscr", bufs=1))

        bc_sb = pp.tile([P, 6, E], F32, name="bc_sb")
        id32_sb = pp.tile([P, P], F32, name="id32_sb")
        id16_sb = pp.tile([P, P], F16, name="id16_sb")
        ones64_sb = pp.tile([1, 64], F32, name="ones64_sb")
        eps_sb = pp.tile([P, 1], F32, name="eps_sb")
        nc.vector.memset(eps_sb[:], 1e-5)
        bq_sb = pp.tile([P, ET], F32, name="bq_sb")
        bk_sb = pp.tile([P, ET], F32, name="bk_sb")
        nc.sync.dma_start(bk_sb[:], bk2[:])
        bv_sb = pp.tile([P, ET], F32, name="bv_sb")
        c1_sb = pp.tile([P, FT], F32, name="c1_sb")
        h_sb = pp.tile([P, QT, E], F32, name="h_sb")
        hT = pp.tile([P, ET, 512], F16, name="hT")

        k_in = dram.tile([E, 512], F16, name="k_in")
        v_in = dram.tile([512, E], F16, name="v_in")
        k_all = dram.tile([4, E, 512], F16, name="k_all")
        v_all = dram.tile([4, 512, E], F16, name="v_all")

        def layer_norm(scratch, y_ap, g_idx, b_idx, out_ap):
            nm = scratch.tile([P, 1], F32, tag="nm")
            nc.vector.reduce_sum(nm[:], y_ap, axis=AX.X)
            nc.vector.tensor_scalar_mul(nm[:], nm[:], -1.0 / E)
            yc = scratch.tile([P, E], F32, tag="yc")
            nc.scalar.activation(yc[:], y_ap, AF.Identity, bias=nm[:])
            sq = scratch.tile([P, E], F32, tag="sq")
            ssq = scratch.tile([P, 1], F32, tag="ssq")
            nc.scalar.activation(sq[:], yc[:], AF.Square, accum_out=ssq[:])
            sd = scratch.tile([P, 1], F32, tag="sd")
            nc.scalar.activation(sd[:], ssq[:], AF.Sqrt, scale=1.0 / E, bias=eps_sb[:])
            rstd = scratch.tile([P, 1], F32, tag="rstd")
            nc.vector.reciprocal(rstd[:], sd[:])
            t1 = scratch.tile([P, E], F32, tag="t1")
            nc.scalar.activation(t1[:], yc[:], AF.Copy, scale=rstd[:])
            t2 = scratch.tile([P, E], F32, tag="t2")
            nc.vector.tensor_mul(t2[:], t1[:], bc_sb[:, g_idx, :])
            nc.vector.tensor_add(out_ap, t2[:], bc_sb[:, b_idx, :])

        # ============ pool scope: x / qT / zT / mask (until end of Wo) ============
        with tc.tile_pool(name="xqz", bufs=1) as xqz:
            x_sb = xqz.tile([P, QT, E], F32, name="x_sb")
            mask_sb = xqz.tile([P, QT, 512], F32, name="mask_sb")
            qT = xqz.tile([P, ET, 512], F16, name="qT")
            zT = xqz.tile([P, ET, 512], F16, name="zT")

            # ---------- phases X/K/V/Q ----------
            with tc.tile_pool(name="qkvbuf", bufs=1) as qkvbuf:
                xT = qkvbuf.tile([P, ET, 512], F16, name="xT")
                for k in range(ET):
                    nc.sync.dma_start(xT[:, k, :], xt16[:, k, :])
                wk_sb = qkvbuf.tile([P, ET, E], F16, name="wk_sb")
                wv_sb = qkvbuf.tile([P, ET, E], F16, name="wv_sb")
                wq_sb = qkvbuf.tile([P, ET, E], F16, name="wq_sb")
                for w_sb, wdram in ((wk_sb, wk), (wv_sb, wv), (wq_sb, wq)):
                    for k in range(ET):
                        nc.sync.dma_start(w_sb[:, k, :], wdram[ts(k, P), :])

                def proj_fm(w_sb, bias_sb, out_sb, qkv_ps):
                    for m in range(ET):
                        ps = qkv_ps.tile([P, 512], F32, tag="qkvps")
                        for k in range(ET):
                            nc.tensor.matmul(
                                ps[:], w_sb[:, k, ts(m, P)], xT[:, k, :],
                                start=(k == 0), stop=(k == ET - 1),
                            )
                        nc.scalar.activation(
                            out_sb[:, m, :], ps[:], AF.Identity, bias=bias_sb[:, m : m + 1]
                        )

                kT = qkvbuf.tile([P, ET, 512], F16, name="kT")
                with tc.tile_pool(name="k_ps", bufs=2, space="PSUM") as k_ps:
                    proj_fm(wk_sb, bk_sb, kT, k_ps)
                nc.sync.dma_start(k_in.rearrange("(m p) t -> p m t", p=P), kT[:])
                if collectives:
                    nc.gpsimd.collective_compute(
                        "AllGather", OP.bypass, replica_groups=GROUPS,
                        ins=[k_in.opt()], outs=[k_all.opt()],
                    )
                else:
                    nc.sync.dma_start(k_all[0], k_in[:])
                # non-critical loads, emitted once the PE pipeline is rolling
                nc.sync.dma_start(mask_sb[:], maskt.rearrange("t p n -> p t n"))
                for eo in range(0, ET, 2):
                    for t in range(QT):
                        nc.sync.dma_start(
                            x_sb[:, t, ts(eo // 2, 256)], xloc[t, :, ts(eo // 2, 256)]
                        )
                nc.sync.dma_start(bc_sb[:], bcast[:])
                nc.sync.dma_start(id32_sb[:], id32[:])
                nc.sync.dma_start(id16_sb[:], id16[:])
                nc.sync.dma_start(ones64_sb[:], ones64[:])
                nc.sync.dma_start(bq_sb[:], bq2[:])
                nc.sync.dma_start(bv_sb[:], bv2[:])
                nc.sync.dma_start(c1_sb[:], c1l[:])

                v_sb = qkvbuf.tile([P, QT, E], F16, name="v_sb")
                with tc.tile_pool(name="v_ps", bufs=1, space="PSUM") as v_ps:
                    vps = [v_ps.tile([P, 512], F32, name=f"vps{i}") for i in range(8)]
                    for k in range(ET):
                        for t in range(QT):
                            for half in range(2):
                                nc.tensor.matmul(
                                    vps[t * 2 + half][:],
                                    xT[:, k, ts(t, P)],
                                    wv_sb[:, k, ts(half, 512)],
                                    start=(k == 0), stop=(k == ET - 1),
                                )
                    for t in range(QT):
                        for half in range(2):
                            nc.vector.tensor_copy(
                                v_sb[:, t, ts(half, 512)], vps[t * 2 + half][:]
                            )
                nc.sync.dma_start(v_in.rearrange("(t p) e -> p t e", p=P), v_sb[:])
                if collectives:
                    nc.gpsimd.collective_compute(
                        "AllGather", OP.bypass, replica_groups=GROUPS,
                        ins=[v_in.opt()], outs=[v_all.opt()],
                    )
                else:
                    nc.sync.dma_start(v_all[0], v_in[:])

                with tc.tile_pool(name="q_ps", bufs=2, space="PSUM") as q_ps:
                    proj_fm(wq_sb, bq_sb, qT, q_ps)

            # ---------- phase A: attention ----------
            with (
                tc.tile_pool(name="kv", bufs=2) as kvpool,
                tc.tile_pool(name="attn", bufs=2) as apool,
                tc.tile_pool(name="small", bufs=4) as spool,
                tc.tile_pool(name="sc_ps", bufs=2, space="PSUM") as sc_ps,
                tc.tile_pool(name="pt_ps", bufs=2, space="PSUM") as pt_ps,
                tc.tile_pool(name="z_ps", bufs=1, space="PSUM") as z_ps,
                tc.tile_pool(name="b_ps", bufs=1, space="PSUM") as b_ps,
            ):
                for m in range(ET):  # head pairs
                    # natural-order caches; 2 strided DMAs per rank segment
                    kT2 = kvpool.tile([P, 16, P], F16, tag="kT2")
                    v2 = kvpool.tile([P, 16, P], F16, tag="v2")
                    for qr in range(4):
                        ksrc = k_all[qr, ts(m, P), :].rearrange("p (u q) -> p u q", q=P)
                        vsrc = v_all[qr, :, ts(m, P)].rearrange("(u p) d -> p u d", p=P)
                        for pa, u0 in ((qr, 0), (7 - qr, 1)):
                            nc.sync.dma_start(
                                kT2[:, pa : pa + 9 : 8, :], ksrc[:, u0::2, :]
                            )
                            nc.sync.dma_start(
                                v2[:, pa : pa + 9 : 8, :], vsrc[:, u0::2, :]
                            )
                    for t in range(QT):
                        ktn = 4 * (t + 1)
                        exps = []
                        sums = []
                        for hh in range(2):
                            exps.append(apool.tile([P, 16, P], F16, tag=f"exp{hh}", name=f"exp{hh}"))
                            sums.append(spool.tile([P, QT], F32, tag=f"sums{hh}", name=f"sums{hh}"))
                        for c in range(t + 1):
                            for hh in range(2):
                                bp = 64 * hh
                                sps = sc_ps.tile([P, 512], F32, tag=f"sps{hh}", name=f"sps{hh}")
                                nc.tensor.matmul(
                                    sps[:],
                                    qT[bp : bp + 64, m, ts(t, P)],
                                    kT2[bp : bp + 64, 4 * c : 4 * c + 4, :],
                                    start=True, stop=True,
                                )
                                if c == t:
                                    nc.vector.tensor_add(sps[:], sps[:], mask_sb[:, t, :])
                                nc.scalar.activation(
                                    exps[hh][:, 4 * c : 4 * c + 4, :], sps[:], AF.Exp,
                                    scale=0.125, accum_out=sums[hh][:, c : c + 1],
                                )
                        zps = z_ps.tile([P, P], F32, tag="zps")
                        bcs2 = [
                            spool.tile([P, P], F32, tag=f"bcs{i}", name=f"bcs{i}")
                            for i in range(2)
                        ]
                        for hh in range(2):
                            bp = 64 * hh
                            pT = apool.tile([P, 16, P], F16, tag=f"pT{hh}", name=f"pT{hh}")
                            for cc in range(t + 1):
                                tps = pt_ps.tile([P, 4, P], F16, tag="tps")
                                for j in range(4):
                                    nc.tensor.transpose(
                                        tps[:, j, :], exps[hh][:, 4 * cc + j, :], id16_sb[:]
                                    )
                                nc.vector.tensor_copy(pT[:, 4 * cc : 4 * cc + 4, :], tps[:])
                            for kt in range(ktn):
                                nc.tensor.matmul(
                                    zps[bp : bp + 64, :],
                                    v2[:, kt, bp : bp + 64],
                                    pT[:, kt, :],
                                    start=(kt == 0), stop=(kt == ktn - 1),
                                    tile_position=(0, bp),
                                )
                            s1 = spool.tile([P, 1], F32, tag=f"s1{hh}", name=f"s1{hh}")
                            nc.vector.reduce_sum(s1[:], sums[hh][:, 0 : t + 1], axis=AX.X)
                            s1T = b_ps.tile([1, P], F32, tag="s1T")
                            nc.tensor.transpose(s1T[:], s1[:], id32_sb[:])
                            recipT = spool.tile([1, P], F32, tag=f"recipT{hh}", name=f"recipT{hh}")
                            nc.vector.reciprocal(recipT[:], s1T[:])
                            nc.gpsimd.partition_broadcast(bcs2[hh][:], recipT[:])
                        for hh in range(2):
                            bp = 64 * hh
                            nc.vector.tensor_mul(
                                zps[bp : bp + 64, :], zps[bp : bp + 64, :],
                                bcs2[hh][bp : bp + 64, :],
                            )
                        nc.scalar.activation(
                            zT[:, m, ts(t, P)], zps[:],
                            AF.Identity, bias=bv_sb[:, m : m + 1],
                        )

            # ---------- phase O: Wo + residual (x last use) ----------
            with (
                tc.tile_pool(name="wo_ps", bufs=1, space="PSUM") as wo_ps,
                tc.tile_pool(name="wos", bufs=3) as wos,
                tc.tile_pool(name="tp2_ps", bufs=2, space="PSUM") as tp2_ps,
            ):
                for pair in range(2):
                    ops = [
                        wo_ps.tile([P, 512], F32, tag=f"ops{i}", name=f"ops{pair}_{i}")
                        for i in range(4)
                    ]
                    for k in range(ET):
                        wt = wos.tile([P, E], F16, tag="wot")
                        nc.sync.dma_start(wt[:], wo[ts(k, P), :])
                        for ti in range(2):
                            t = 2 * pair + ti
                            for half in range(2):
                                nc.tensor.matmul(
                                    ops[2 * ti + half][:], zT[:, k, ts(t, P)],
                                    wt[:, ts(half, 512)],
                                    start=(k == 0), stop=(k == ET - 1),
                                )
                    for ti in range(2):
                        t = 2 * pair + ti
                        for half in range(2):
                            nc.vector.tensor_add(
                                ops[2 * ti + half][:], ops[2 * ti + half][:],
                                x_sb[:, t, ts(half, 512)],
                            )
                            nc.vector.tensor_add(
                                h_sb[:, t, ts(half, 512)], ops[2 * ti + half][:],
                                bc_sb[:, 0, ts(half, 512)],
                            )
                        layer_norm(lnscr, h_sb[:, t, :], 2, 3, h_sb[:, t, :])
                        for eo in range(ET):
                            tp = tp2_ps.tile([P, P], F32, tag="tp2")
                            nc.tensor.transpose(tp[:], h_sb[:, t, ts(eo, P)], id32_sb[:])
                            nc.scalar.copy(hT[:, eo, ts(t, P)], tp[:])
        # xqz closed: x/qT/zT/mask freed

        # ---------- F1 + F2 ----------
        with tc.tile_pool(name="ffbuf", bufs=1) as ffbuf:
            ff1T = ffbuf.tile([P, FT, 512], F16, name="ff1T")
            w2t = ffbuf.tile([P, FT, E], F16, name="w2t")
            nc.sync.dma_start(w2t[:], w2.rearrange("(ko p) n -> p ko n", p=P))
            with (
                tc.tile_pool(name="wos2", bufs=6) as wos2,
            ):
                with tc.tile_pool(name="f1_ps", bufs=2, space="PSUM") as f1_ps:
                    for mf in range(FT):
                        wt = wos2.tile([P, ET, P], F16, tag="w1t")
                        nc.sync.dma_start(
                            wt[:], w1[:, ts(mf, P)].rearrange("(ko p) n -> p ko n", p=P)
                        )
                        ps = f1_ps.tile([P, 512], F32, tag="f1ps")
                        for k in range(ET):
                            nc.tensor.matmul(
                                ps[:], wt[:, k, :], hT[:, k, :],
                                start=(k == 0), stop=(k == ET - 1),
                            )
                        nc.scalar.activation(
                            ff1T[:, mf, :], ps[:], AF.Relu, bias=c1_sb[:, mf : mf + 1]
                        )

            with (
                tc.tile_pool(name="f2_ps", bufs=1, space="PSUM") as f2_ps,
                tc.tile_pool(name="outp", bufs=2) as out_pool,
            ):
                for t in range(QT):
                    f2s = [
                        f2_ps.tile([P, 512], F32, tag=f"f2h{half}", bufs=2,
                                   name=f"f2s{t}_{half}")
                        for half in range(2)
                    ]
                    for k in range(FT):
                        for half in range(2):
                            nc.tensor.matmul(
                                f2s[half][:],
                                ff1T[:, k, ts(t, P)],
                                w2t[:, k, ts(half, 512)],
                                start=(k == 0), stop=(k == FT - 1),
                            )
                    for half in range(2):
                        nc.vector.tensor_add(
                            f2s[half][:], f2s[half][:], h_sb[:, t, ts(half, 512)]
                        )
                        nc.vector.tensor_add(
                            h_sb[:, t, ts(half, 512)], f2s[half][:],
                            bc_sb[:, 1, ts(half, 512)],
                        )
                    o_sb = out_pool.tile([P, E], F32, tag="o")
                    layer_norm(lnscr, h_sb[:, t, :], 4, 5, o_sb[:])
                    nc.sync.dma_start(yloc[t], o_sb[:])

    nc.compile()
    return nc


_PROG = None


def _get_program():
    global _PROG
    if _PROG is None:
        _PROG = _build_program()
    return _PROG


def _prep_inputs(x, Wq, bq, Wk, bk, Wv, bv, Wo, bo, W1, c1, W2, c2, g1, beta1, g2, beta2):
    f32 = lambda a: np.ascontiguousarray(np.asarray(a), dtype=np.float32)
    x = f32(x)
    wq = f32(Wq).transpose(1, 0, 2).reshape(E, E).astype(np.float16)
    wk = f32(Wk).transpose(1, 0, 2).reshape(E, E).astype(np.float16)
    wv = f32(Wv).transpose(1, 0, 2).reshape(E, E).astype(np.float16)
    wo = f32(Wo).astype(np.float16)
    w1 = f32(W1).astype(np.float16)
    w2 = f32(W2).astype(np.float16)
    fm = lambda v, nt: np.ascontiguousarray(f32(v).reshape(nt, P).T)
    bq2, bk2, bv2 = fm(bq, ET), fm(bk, ET), fm(bv, ET)
    c1l = fm(c1, FT)
    bcast = np.ascontiguousarray(
        np.broadcast_to(
            np.stack([f32(bo), f32(c2), f32(g1), f32(beta1), f32(g2), f32(beta2)]),
            (P, 6, E),
        )
    )
    id32 = np.eye(P, dtype=np.float32)
    id16 = np.eye(P, dtype=np.float16)
    ones64 = np.ones((1, 64), dtype=np.float32)

    common = dict(
        wq=wq, wk=wk, wv=wv, wo=wo, w1=w1, w2=w2,
        bq2=bq2, bk2=bk2, bv2=bv2, c1l=c1l, bcast=bcast,
        id32=id32, id16=id16, ones64=ones64,
    )
    in_maps = []
    for r in range(NCORE):
        beta, q = divmod(r, 4)
        bm = _bmap(q)
        xl = np.stack([x[beta, 128 * b : 128 * b + 128, :] for b in bm])
        mk = np.zeros((QT, P, 512), dtype=np.float32)
        for t, b in enumerate(bm):
            kk = 512 * t + np.arange(512)[None, :]
            valid = kk <= (128 * b + np.arange(P)[:, None])
            mk[t] = np.where(valid, 0.0, -1e9)
        m = dict(common)
        m["xloc"] = np.ascontiguousarray(xl)
        m["xt16"] = np.ascontiguousarray(
            xl.reshape(QT, P, ET, P).transpose(3, 2, 0, 1).reshape(P, ET, QT * P)
        ).astype(np.float16)
        m["maskt"] = mk
        in_maps.append(m)
    return in_maps


def _assemble(results):
    y = np.empty((B, L, E), dtype=np.float32)
    for r in range(NCORE):
        beta, q = divmod(r, 4)
        yl = results[r]["yloc"]
        for t, b in enumerate(_bmap(q)):
            y[beta, 128 * b : 128 * b + 128, :] = yl[t]
    return y


def kernel(**inputs):
    inputs = {k: v for k, v in inputs.items() if k != "mask"}
    nc = _get_program()
    in_maps = _prep_inputs(**inputs)
    res = run_bass_kernel_spmd(nc, in_maps, core_ids=list(range(NCORE)))
    kernel.last_results = res
    return _assemble(res.results)


if __name__ == "__main__":
    rng = np.random.default_rng(0)
    print("building program...")
    _get_program()
    print("built ok")


# revision 34
# speedup vs baseline: 160.0776x; 160.0776x over previous
"""Trainium2 Bass kernel for a dense transformer decoder layer.

B=2, L=2048, E=1024, H=16 (Dh=64), Dff=4096, fp32 I/O.

Strategy (8 NeuronCores):
  - Sequence parallel: B*L = 4096 tokens sharded 512 rows/core using a
    zigzag assignment of 128-row blocks {q, 7-q, 8+q, 15-q} within each
    batch (4 cores per batch) so causal attention work is balanced and
    the program is identical on every core (q-tile key extents padded to
    the uniform {512, 1024, 1536, 2048}; the per-core causal boundary is
    data-driven via an additive -1e9 mask on the diagonal 512-chunk).
  - Each core computes K,V for its own rows (all heads), AllGathers them
    in fp16 within its batch group ([[0..3],[4..7]]), then runs causal
    attention for its own query rows, followed by Wo, LN1, FF, LN2 —
    all fully local (no all-reduce anywhere).
  - All big matmuls run with fp16 operands (host-cast weights, fp16
    activations feature-major) at 1 PE cycle/row, fp32 PSUM accumulation;
    softmax and LayerNorm math stays fp32. x arrives both row-major fp32
    (residual path) and feature-major fp16 (host-marshalled xT input, so
    no on-device transpose of x is needed).
"""

import sys

if "/opt/trn_rl_repo" not in sys.path:
    sys.path.insert(0, "/opt/trn_rl_repo")

from contextlib import ExitStack

import numpy as np

import concourse.bass as bass
import concourse.mybir as mybir
from concourse import bacc
from concourse.bass import ts
from concourse.bass_utils import run_bass_kernel_spmd
from concourse.tile import TileContext

B, L, E, H, Dh, Dff = 2, 2048, 1024, 16, 64, 4096
P = 128
ET = E // P            # 8 feature tiles
FT = Dff // P          # 32 ff tiles
QT = 4                 # q-tiles (128 rows) per core
NCORE = 8
GROUPS = [[0, 1, 2, 3], [4, 5, 6, 7]]
F32, F32R, F16 = mybir.dt.float32, mybir.dt.float32r, mybir.dt.float16
AF = mybir.ActivationFunctionType
OP = mybir.AluOpType
AX = mybir.AxisListType


def _bmap(q):
    return [q, 7 - q, 8 + q, 15 - q]


def _owner(nb):
    if nb < 4:
        return nb, 0
    if nb < 8:
        return 7 - nb, 1
    if nb < 12:
        return nb - 8, 2
    return 15 - nb, 3


def _round_f32r(a):
    b = np.ascontiguousarray(a, np.float32).view(np.uint32)
    r = (b + np.uint32(0x7FF) + ((b >> np.uint32(12)) & np.uint32(1))) & np.uint32(0xFFFFF000)
    return r.view(np.float32)


def _build_program(collectives=True):
    nc = bacc.Bacc("TRN2", target_bir_lowering=False, debug=False, num_devices=NCORE)

    xloc = nc.dram_tensor("xloc", [QT, P, E], F32, kind="ExternalInput")
    xt16 = nc.dram_tensor("xt16", [P, ET, 512], F16, kind="ExternalInput")
    wq = nc.dram_tensor("wq", [E, E], F16, kind="ExternalInput")
    wk = nc.dram_tensor("wk", [E, E], F16, kind="ExternalInput")
    wv = nc.dram_tensor("wv", [E, E], F16, kind="ExternalInput")
    wo = nc.dram_tensor("wo", [E, E], F16, kind="ExternalInput")
    w1 = nc.dram_tensor("w1", [E, Dff], F16, kind="ExternalInput")
    w2 = nc.dram_tensor("w2", [Dff, E], F16, kind="ExternalInput")
    bq2 = nc.dram_tensor("bq2", [P, ET], F32, kind="ExternalInput")
    bk2 = nc.dram_tensor("bk2", [P, ET], F32, kind="ExternalInput")
    bv2 = nc.dram_tensor("bv2", [P, ET], F32, kind="ExternalInput")
    c1l = nc.dram_tensor("c1l", [P, FT], F32, kind="ExternalInput")
    bcast = nc.dram_tensor("bcast", [P, 6, E], F32, kind="ExternalInput")
    maskt = nc.dram_tensor("maskt", [QT, P, 512], F32, kind="ExternalInput")
    id32 = nc.dram_tensor("id32", [P, P], F32, kind="ExternalInput")
    id16 = nc.dram_tensor("id16", [P, P], F16, kind="ExternalInput")
    ones64 = nc.dram_tensor("ones64", [1, 64], F32, kind="ExternalInput")
    yloc = nc.dram_tensor("yloc", [QT, P, E], F32, kind="ExternalOutput")

    with TileContext(nc) as tc, ExitStack() as ctx:
        pp = ctx.enter_context(tc.tile_pool(name="persist", bufs=1))
        dram = ctx.enter_context(tc.tile_pool(name="dram", bufs=1, space="DRAM"))
        lnscr = ctx.enter_context(tc.tile_pool(name="lnscr", bufs=1))

        bc_sb = pp.tile([P, 6, E], F32, name="bc_sb")
        id32_sb = pp.tile([P, P], F32, name="id32_sb")
        id16_sb = pp.tile([P, P], F16, name="id16_sb")
        ones64_sb = pp.tile([1, 64], F32, name="ones64_sb")
        eps_sb = pp.tile([P, 1], F32, name="eps_sb")
        nc.vector.memset(eps_sb[:], 1e-5)
        bq_sb = pp.tile([P, ET], F32, name="bq_sb")
        bk_sb = pp.tile([P, ET], F32, name="bk_sb")
        nc.sync.dma_start(bk_sb[:], bk2[:])
        bv_sb = pp.tile([P, ET], F32, name="bv_sb")
        c1_sb = pp.tile([P, FT], F32, name="c1_sb")
        h_sb = pp.tile([P, QT, E], F32, name="h_sb")
        hT = pp.tile([P, ET, 512], F16, name="hT")

        k_in = dram.tile([E, 512], F16, name="k_in")
        v_in = dram.tile([512, E], F16, name="v_in")
        k_all = dram.tile([4, E, 512], F16, name="k_all")
        v_all = dram.tile([4, 512, E], F16, name="v_all")

        def layer_norm(scratch, y_ap, g_idx, b_idx, out_ap):
            nm = scratch.tile([P, 1], F32, tag="nm")
            nc.vector.reduce_sum(nm[:], y_ap, axis=AX.X)
            nc.vector.tensor_scalar_mul(nm[:], nm[:], -1.0 / E)
            yc = scratch.tile([P, E], F32, tag="yc")
            nc.scalar.activation(yc[:], y_ap, AF.Identity, bias=nm[:])
            sq = scratch.tile([P, E], F32, tag="sq")
            ssq = scratch.tile([P, 1], F32, tag="ssq")
            nc.scalar.activation(sq[:], yc[:], AF.Square, accum_out=ssq[:])
            sd = scratch.tile([P, 1], F32, tag="sd")
            nc.scalar.activation(sd[:], ssq[:], AF.Sqrt, scale=1.0 / E, bias=eps_sb[:])
            rstd = scratch.tile([P, 1], F32, tag="rstd")
            nc.vector.reciprocal(rstd[:], sd[:])
            t1 = scratch.tile([P, E], F32, tag="t1")
            nc.scalar.activation(t1[:], yc[:], AF.Copy, scale=rstd[:])
            t2 = scratch.tile([P, E], F32, tag="t2")
            nc.vector.tensor_mul(t2[:], t1[:], bc_sb[:, g_idx, :])
            nc.vector.tensor_add(out_ap, t2[:], bc_sb[:, b_idx, :])

        # ============ pool scope: x / qT / zT / mask (until end of Wo) ============
        with tc.tile_pool(name="xqz", bufs=1) as xqz:
            x_sb = xqz.tile([P, QT, E], F32, name="x_sb")
            mask_sb = xqz.tile([P, QT, 512], F32, name="mask_sb")
            qT = xqz.tile([P, ET, 512], F16, name="qT")
            zT = xqz.tile([P, ET, 512], F16, name="zT")

            # ---------- phases X/K/V/Q ----------
            with tc.tile_pool(name="qkvbuf", bufs=1) as qkvbuf:
                xT = qkvbuf.tile([P, ET, 512], F16, name="xT")
                for k in range(ET):
                    nc.sync.dma_start(xT[:, k, :], xt16[:, k, :])
                wk_sb = qkvbuf.tile([P, ET, E], F16, name="wk_sb")
                wv_sb = qkvbuf.tile([P, ET, E], F16, name="wv_sb")
                wq_sb = qkvbuf.tile([P, ET, E], F16, name="wq_sb")
                for w_sb, wdram in ((wk_sb, wk), (wv_sb, wv), (wq_sb, wq)):
                    for k in range(ET):
                        nc.sync.dma_start(w_sb[:, k, :], wdram[ts(k, P), :])

                def proj_fm(w_sb, bias_sb, out_sb, qkv_ps):
                    for m in range(ET):
                        ps = qkv_ps.tile([P, 512], F32, tag="qkvps")
                        for k in range(ET):
                            nc.tensor.matmul(
                                ps[:], w_sb[:, k, ts(m, P)], xT[:, k, :],
                                start=(k == 0), stop=(k == ET - 1),
                            )
                        nc.scalar.activation(
                            out_sb[:, m, :], ps[:], AF.Identity, bias=bias_sb[:, m : m + 1]
                        )

                kT = qkvbuf.tile([P, ET, 512], F16, name="kT")
                with tc.tile_pool(name="k_ps", bufs=2, space="PSUM") as k_ps:
                    proj_fm(wk_sb, bk_sb, kT, k_ps)
                nc.sync.dma_start(k_in.rearrange("(m p) t -> p m t", p=P), kT[:])
                if collectives:
                    nc.gpsimd.collective_compute(
                        "AllGather", OP.bypass, replica_groups=GROUPS,
                        ins=[k_in.opt()], outs=[k_all.opt()],
                    )
                else:
                    nc.sync.dma_start(k_all[0], k_in[:])
                # non-critical loads, emitted once the PE pipeline is rolling
                nc.sync.dma_start(mask_sb[:], maskt.rearrange("t p n -> p t n"))
                for eo in range(0, ET, 2):
                    for t in range(QT):
                        nc.sync.dma_start(
                            x_sb[:, t, ts(eo // 2, 256)], xloc[t, :, ts(eo // 2, 256)]
                        )
                nc.sync.dma_start(bc_sb[:], bcast[:])
                nc.sync.dma_start(id32_sb[:], id32[:])
                nc.sync.dma_start(id16_sb[:], id16[:])
                nc.sync.dma_start(ones64_sb[:], ones64[:])
                nc.sync.dma_start(bq_sb[:], bq2[:])
                nc.sync.dma_start(bv_sb[:], bv2[:])
                nc.sync.dma_start(c1_sb[:], c1l[:])

                v_sb = qkvbuf.tile([P, QT, E], F16, name="v_sb")
                with tc.tile_pool(name="v_ps", bufs=1, space="PSUM") as v_ps:
                    vps = [v_ps.tile([P, 512], F32, name=f"vps{i}") for i in range(8)]
                    for k in range(ET):
                        for t in range(QT):
                            for half in range(2):
                                nc.tensor.matmul(
                                    vps[t * 2 + half][:],
                                    xT[:, k, ts(t, P)],
                                    wv_sb[:, k, ts(half, 512)],
                                    start=(k == 0), stop=(k == ET - 1),
                                )
                    for t in range(QT):
                        for half in range(2):
                            nc.vector.tensor_copy(
                                v_sb[:, t, ts(half, 512)], vps[t * 2 + half][:]
                            )
                nc.sync.dma_start(v_in.rearrange("(t p) e -> p t e", p=P), v_sb[:])
                if collectives:
                    nc.gpsimd.collective_compute(
                        "AllGather", OP.bypass, replica_groups=GROUPS,
                        ins=[v_in.opt()], outs=[v_all.opt()],
                    )
                else:
                    nc.sync.dma_start(v_all[0], v_in[:])

                with tc.tile_pool(name="q_ps", bufs=2, space="PSUM") as q_ps:
                    proj_fm(wq_sb, bq_sb, qT, q_ps)

            # ---------- phase A: attention ----------
            with (
                tc.tile_pool(name="kv", bufs=2) as kvpool,
                tc.tile_pool(name="attn", bufs=2) as apool,
                tc.tile_pool(name="small", bufs=4) as spool,
                tc.tile_pool(name="sc_ps", bufs=2, space="PSUM") as sc_ps,
                tc.tile_pool(name="pt_ps", bufs=2, space="PSUM") as pt_ps,
                tc.tile_pool(name="z_ps", bufs=1, space="PSUM") as z_ps,
                tc.tile_pool(name="b_ps", bufs=1, space="PSUM") as b_ps,
            ):
                for m in range(ET):  # head pairs
                    # natural-order caches; 2 strided DMAs per rank segment
                    kT2 = kvpool.tile([P, 16, P], F16, tag="kT2")
                    v2 = kvpool.tile([P, 16, P], F16, tag="v2")
                    for qr in range(4):
                        ksrc = k_all[qr, ts(m, P), :].rearrange("p (u q) -> p u q", q=P)
                        vsrc = v_all[qr, :, ts(m, P)].rearrange("(u p) d -> p u d", p=P)
                        for pa, u0 in ((qr, 0), (7 - qr, 1)):
                            nc.sync.dma_start(
                                kT2[:, pa : pa + 9 : 8, :], ksrc[:, u0::2, :]
                            )
                            nc.sync.dma_start(
                                v2[:, pa : pa + 9 : 8, :], vsrc[:, u0::2, :]
                            )
                    for t in range(QT):
                        ktn = 4 * (t + 1)
                        exps = []
                        sums = []
                        for hh in range(2):
                            exps.append(apool.tile([P, 16, P], F16, tag=f"exp{hh}", name=f"exp{hh}"))
                            sums.append(spool.tile([P, QT], F32, tag=f"sums{hh}", name=f"sums{hh}"))
                        for c in range(t + 1):
                            for hh in range(2):
                                bp = 64 * hh
                                sps = sc_ps.tile([P, 512], F32, tag=f"sps{hh}", name=f"sps{hh}")
                                nc.tensor.matmul(
                                    sps[:],
                                    qT[bp : bp + 64, m, ts(t, P)],
                                    kT2[bp : bp + 64, 4 * c : 4 * c + 4, :],
                                    start=True, stop=True,
                                )
                                if c == t:
                                    nc.vector.tensor_add(sps[:], sps[:], mask_sb[:, t, :])
                                nc.scalar.activation(
                                    exps[hh][:, 4 * c : 4 * c + 4, :], sps[:], AF.Exp,
                                    scale=0.125, accum_out=sums[hh][:, c : c + 1],
                                )
                        zps = z_ps.tile([P, P], F32, tag="zps")
                        bcs2 = [
                            spool.tile([P, P], F32, tag=f"bcs{i}", name=f"bcs{i}")
                            for i in range(2)
                        ]
                        for hh in range(2):
                            bp = 64 * hh
                            pT = apool.tile([P, 16, P], F16, tag=f"pT{hh}", name=f"pT{hh}")
                            for cc in range(t + 1):
                                tps = pt_ps.tile([P, 4, P], F16, tag="tps")
                                for j in range(4):
                                    nc.tensor.transpose(
                                        tps[:, j, :], exps[hh][:, 4 * cc + j, :], id16_sb[:]
                                    )
                                nc.vector.tensor_copy(pT[:, 4 * cc : 4 * cc + 4, :], tps[:])
                            for kt in range(ktn):
                                nc.tensor.matmul(
                                    zps[bp : bp + 64, :],
                                    v2[:, kt, bp : bp + 64],
                                    pT[:, kt, :],
                                    start=(kt == 0), stop=(kt == ktn - 1),
                                    tile_position=(0, bp),
                                )
                            s1 = spool.tile([P, 1], F32, tag=f"s1{hh}", name=f"s1{hh}")
                            nc.vector.reduce_sum(s1[:], sums[hh][:, 0 : t + 1], axis=AX.X)
                            s1T = b_ps.tile([1, P], F32, tag="s1T")
                            nc.tensor.transpose(s1T[:], s1[:], id32_sb[:])
                            recipT = spool.tile([1, P], F32, tag=f"recipT{hh}", name=f"recipT{hh}")
                            nc.vector.reciprocal(recipT[:], s1T[:])
                            nc.gpsimd.partition_broadcast(bcs2[hh][:], recipT[:])
                        for hh in range(2):
                            bp = 64 * hh
                            nc.vector.tensor_mul(
                                zps[bp : bp + 64, :], zps[bp : bp + 64, :],
                                bcs2[hh][bp : bp + 64, :],
                            )
                        nc.scalar.activation(
                            zT[:, m, ts(t, P)], zps[:],
                            AF.Identity, bias=bv_sb[:, m : m + 1],
                        )

            # ---------- phase O: Wo + residual (x last use) ----------
            with (
                tc.tile_pool(name="wo_ps", bufs=1, space="PSUM") as wo_ps,
                tc.tile_pool(name="wos", bufs=3) as wos,
                tc.tile_pool(name="tp2_ps", bufs=2, space="PSUM") as tp2_ps,
            ):
                for pair in range(2):
                    ops = [
                        wo_ps.tile([P, 512], F32, tag=f"ops{i}", name=f"ops{pair}_{i}")
                        for i in range(4)
                    ]
                    for k in range(ET):
                        wt = wos.tile([P, E], F16, tag="wot")
                        nc.sync.dma_start(wt[:], wo[ts(k, P), :])
                        for ti in range(2):
                            t = 2 * pair + ti
                            for half in range(2):
                                nc.tensor.matmul(
                                    ops[2 * ti + half][:], zT[:, k, ts(t, P)],
                                    wt[:, ts(half, 512)],
                                    start=(k == 0), stop=(k == ET - 1),
                                )
                    for ti in range(2):
                        t = 2 * pair + ti
                        for half in range(2):
                            nc.vector.tensor_add(
                                ops[2 * ti + half][:], ops[2 * ti + half][:],
                                x_sb[:, t, ts(half, 512)],
                            )
                            nc.vector.tensor_add(
                                h_sb[:, t, ts(half, 512)], ops[2 * ti + half][:],
                                bc_sb[:, 0, ts(half, 512)],
                            )
                        layer_norm(lnscr, h_sb[:, t, :], 2, 3, h_sb[:, t, :])
                        for eo in range(ET):
                            tp = tp2_ps.tile([P, P], F32, tag="tp2")
                            nc.tensor.transpose(tp[:], h_sb[:, t, ts(eo, P)], id32_sb[:])
                            nc.scalar.copy(hT[:, eo, ts(t, P)], tp[:])
        # xqz closed: x/qT/zT/mask freed

        # ---------- F1 + F2 ----------
        with tc.tile_pool(name="ffbuf", bufs=1) as ffbuf:
            ff1T = ffbuf.tile([P, FT, 512], F16, name="ff1T")
            w2t = ffbuf.tile([P, FT, E], F16, name="w2t")
            nc.sync.dma_start(w2t[:], w2.rearrange("(ko p) n -> p ko n", p=P))
            with (
                tc.tile_pool(name="wos2", bufs=6) as wos2,
            ):
                with tc.tile_pool(name="f1_ps", bufs=2, space="PSUM") as f1_ps:
                    for mf in range(FT):
                        wt = wos2.tile([P, ET, P], F16, tag="w1t")
                        nc.sync.dma_start(
                            wt[:], w1[:, ts(mf, P)].rearrange("(ko p) n -> p ko n", p=P)
                        )
                        ps = f1_ps.tile([P, 512], F32, tag="f1ps")
                        for k in range(ET):
                            nc.tensor.matmul(
                                ps[:], wt[:, k, :], hT[:, k, :],
                                start=(k == 0), stop=(k == ET - 1),
                            )
                        nc.scalar.activation(
                            ff1T[:, mf, :], ps[:], AF.Relu, bias=c1_sb[:, mf : mf + 1]
                        )

            with (
                tc.tile_pool(name="f2_ps", bufs=1, space="PSUM") as f2_ps,
                tc.tile_pool(name="outp", bufs=2) as out_pool,
            ):
                for t in range(QT):
                    f2s = [
                        f2_ps.tile([P, 512], F32, tag=f"f2h{half}", bufs=2,
                                   name=f"f2s{t}_{half}")
                        for half in range(2)
                    ]
                    for k in range(FT):
                        for half in range(2):
                            nc.tensor.matmul(
                                f2s[half][:],
                                ff1T[:, k, ts(t, P)],
                                w2t[:, k, ts(half, 512)],
                                start=(k == 0), stop=(k == FT - 1),
                            )
                    for half in range(2):
                        nc.vector.tensor_add(
                            f2s[half][:], f2s[half][:], h_sb[:, t, ts(half, 512)]
                        )
                        nc.vector.tensor_add(
                            h_sb[:, t, ts(half, 512)], f2s[half][:],
                            bc_sb[:, 1, ts(half, 512)],
                        )
                    o_sb = out_pool.tile([P, E], F32, tag="o")
                    layer_norm(lnscr, h_sb[:, t, :], 4, 5, o_sb[:])
                    nc.sync.dma_start(yloc[t], o_sb[:])

    nc.compile()
    return nc


_PROG = None


def _get_program():
    global _PROG
    if _PROG is None:
        _PROG = _build_program()
    return _PROG


def _prep_inputs(x, Wq, bq, Wk, bk, Wv, bv, Wo, bo, W1, c1, W2, c2, g1, beta1, g2, beta2):
    f32 = lambda a: np.ascontiguousarray(np.asarray(a), dtype=np.float32)
    x = f32(x)
    wq = f32(Wq).transpose(1, 0, 2).reshape(E, E).astype(np.float16)
    wk = f32(Wk).transpose(1, 0, 2).reshape(E, E).astype(np.float16)
    wv = f32(Wv).transpose(1, 0, 2).reshape(E, E).astype(np.float16)
    wo = f32(Wo).astype(np.float16)
    w1 = f32(W1).astype(np.float16)
    w2 = f32(W2).astype(np.float16)
    fm = lambda v, nt: np.ascontiguousarray(f32(v).reshape(nt, P).T)
    bq2, bk2, bv2 = fm(bq, ET), fm(bk, ET), fm(bv, ET)
    c1l = fm(c1, FT)
    bcast = np.ascontiguousarray(
        np.broadcast_to(
            np.stack([f32(bo), f32(c2), f32(g1), f32(beta1), f32(g2), f32(beta2)]),
            (P, 6, E),
        )
    )
    id32 = np.eye(P, dtype=np.float32)
    id16 = np.eye(P, dtype=np.float16)
    ones64 = np.ones((1, 64), dtype=np.float32)

    common = dict(
        wq=wq, wk=wk, wv=wv, wo=wo, w1=w1, w2=w2,
        bq2=bq2, bk2=bk2, bv2=bv2, c1l=c1l, bcast=bcast,
        id32=id32, id16=id16, ones64=ones64,
    )
    in_maps = []
    for r in range(NCORE):
        beta, q = divmod(r, 4)
        bm = _bmap(q)
        xl = np.stack([x[beta, 128 * b : 128 * b + 128, :] for b in bm])
        mk = np.zeros((QT, P, 512), dtype=np.float32)
        for t, b in enumerate(bm):
            kk = 512 * t + np.arange(512)[None, :]
            valid = kk <= (128 * b + np.arange(P)[:, None])
            mk[t] = np.where(valid, 0.0, -1e9)
        m = dict(common)
        m["xloc"] = np.ascontiguousarray(xl)
        m["xt16"] = np.ascontiguousarray(
            xl.reshape(QT, P, ET, P).transpose(3, 2, 0, 1).reshape(P, ET, QT * P)
        ).astype(np.float16)
        m["maskt"] = mk
        in_maps.append(m)
    return in_maps


def _assemble(results):
    y = np.empty((B, L, E), dtype=np.float32)
    for r in range(NCORE):
        beta, q = divmod(r, 4)
        yl = results[r]["yloc"]
        for t, b in enumerate(_bmap(q)):
            y[beta, 128 * b : 128 * b + 128, :] = yl[t]
    return y


def kernel(**inputs):
    inputs = {k: v for k, v in inputs.items() if k != "mask"}
    nc = _get_program()
    in_maps = _prep_inputs(**inputs)
    res = run_bass_kernel_spmd(nc, in_maps, core_ids=list(range(NCORE)))
    kernel.last_results = res
    return _assemble(res.results)


if __name__ == "__main__":
    rng = np.random.default_rng(0)
    print("building program...")
    _get_program()
    print("built ok")
